# revision 3
# baseline (speedup 1.0000x reference)
"""Trainium2 Bass kernel for a 2-layer masked (ragged) Elman RNN — v2.

Problem: tokens [128,512] -> emb lookup [B,T,1024] -> RNN(1024->2048) ->
RNN(2048->2048) -> final hidden of layer 1, with per-sequence lengths
freezing the hidden state at t >= len (packed-sequence semantics).

Strategy (8 NeuronCores, data-parallel over batch, 16 seqs/core):
  Phase A: embedding gather + bulk input projection xp0 = X@W_ih0 + b0
  Phase B: layer-0 recurrence (512 serial steps)
  Phase C: bulk xp1 = y0 @ W_ih1 + b1 in chunks of 8 timesteps
  Phase D: layer-1 recurrence, storing h1 per step to DRAM; final
           per-sequence capture via indirect gather at t = len-1.

v2 changes vs baseline:
- The recurrence matmuls have M=16 (batch per core), using only 16 of the
  PE's 128 columns. They are now issued to the four 32-column PE tile
  groups (tile_position=(0,32j)): group j computes output n-chunk j with
  the full k accumulation, so four 512-col W streams run concurrently.
  Col-tiling requires 16-bit operands (fp32r matmuls fail the ISA check),
  so the whole matmul datapath is fp16 (same 10-bit mantissa as TF32;
  PSUM accumulation stays fp32).
- Pre-activations land in one PSUM bank in a "grouped" layout: partition
  32j+b holds pre[b, perm(512j+n')]. W columns are host-permuted (swap of
  the two 32-col block fields within each 512 chunk) so the per-step
  hidden-state transposition becomes 16 contiguous [32,128] DVE 32x32
  stream-transposes — entirely off the PE.
- xp tensors are stored in DRAM pre-swizzled to the grouped layout.

The recurrences run unmasked: for t < len the unmasked state equals the
reference's frozen-state values, and the capture row only reads t=len-1.
"""

import sys

sys.path.insert(0, "/opt/trn_rl_repo")

import numpy as np

B, T, V, D, H = 128, 512, 32000, 1024, 2048
NC = 8
BL = B // NC          # 16 sequences per core
KT = H // 128         # 16 k-tiles of the hidden dim
NT = H // 512         # 4 n-tiles (PSUM bank width)
DKT = D // 128        # 8 k-tiles of the embedding dim
CH = 128 // BL        # 8 timesteps per phase-A/C GEMM tile

STATS = {}
_CACHE = {}


def _build(t_steps):
    import concourse.bass as bass
    import concourse.mybir as mybir
    import concourse.tile as tile
    from concourse import bacc
    from concourse.masks import make_identity

    f32 = mybir.dt.float32
    f16 = mybir.dt.float16
    i32 = mybir.dt.int32
    Tanh = mybir.ActivationFunctionType.Tanh

    mt = (t_steps * BL) // 128   # phase-A tile count
    nchunk = t_steps // CH       # phase-C chunk count

    nc = bacc.Bacc("TRN2", target_bir_lowering=False, debug=False, num_devices=NC)

    tokT = nc.dram_tensor("tokT", [128, mt], i32, kind="ExternalInput")
    cap_idx = nc.dram_tensor("cap_idx", [4 * BL, 1], i32, kind="ExternalInput")
    emb = nc.dram_tensor("emb", [V, D], f16, kind="ExternalInput")
    w_ih0 = nc.dram_tensor("w_ih0", [D, H], f16, kind="ExternalInput")
    w_hh0 = nc.dram_tensor("w_hh0", [H, H], f16, kind="ExternalInput")
    b0 = nc.dram_tensor("b0", [1, H], f32, kind="ExternalInput")
    w_ih1 = nc.dram_tensor("w_ih1", [H, H], f16, kind="ExternalInput")
    w_hh1 = nc.dram_tensor("w_hh1", [H, H], f16, kind="ExternalInput")
    b1 = nc.dram_tensor("b1", [1, H], f32, kind="ExternalInput")
    out_h = nc.dram_tensor("out_h", [4 * BL, 512], f16, kind="ExternalOutput")

    # compact grouped layout: [t, 16*j + b, n'] = value[b, perm(512*j + n')]
    xp0_d = nc.dram_tensor("xp0_d", [t_steps, 4 * BL, 512], f16)
    xp1_d = nc.dram_tensor("xp1_d", [t_steps, 4 * BL, 512], f16)
    # per-step transposed state, 32-wide slots: [t, p, 32*k + b] (b < 16)
    y0T_d = nc.dram_tensor("y0T_d", [t_steps, 128, 512], f16)
    # layer-1 h per step, grouped rows (32*j + b valid for b < 16)
    h1_d = nc.dram_tensor("h1_d", [t_steps * 128, 512], f16)

    def load_w(W_sb, wsrc, ktiles):
        # W_sb col block (k*NT+n)*512 holds wsrc[k*128:(k+1)*128, n*512:(n+1)*512]
        for k in range(ktiles):
            nc.gpsimd.dma_start(
                W_sb[:, k * H:(k + 1) * H],
                wsrc[k * 128:(k + 1) * 128, :],
            )

    def load_bias(bias_sb, bsrc):
        nc.gpsimd.dma_start(bias_sb[0:1, :], bsrc[0:1, :])
        nc.gpsimd.partition_broadcast(bias_sb[:], bias_sb[0:1, :])

    with tile.TileContext(nc) as tc:
        with (
            tc.tile_pool(name="wpool", bufs=1) as wp,
            tc.tile_pool(name="state", bufs=1) as st,
        ):
            W_sb = wp.tile([128, KT * H], f16)       # 64KB/partition
            ident = st.tile([128, 128], f16)
            make_identity(nc, ident[:])
            bias_sb = st.tile([128, H], f32)
            tokens_sb = st.tile([128, mt], i32)
            nc.gpsimd.dma_start(tokens_sb[:], tokT[:, :])

            # ---------------- Phase A: embed + xp0 ----------------
            load_w(W_sb, w_ih0, DKT)
            load_bias(bias_sb, b0)
            with (
                nc.named_scope("phaseA"),
                tc.tile_pool(name="ga", bufs=3) as gp,
                tc.tile_pool(name="xt", bufs=2) as xtp,
                tc.tile_pool(name="pa", bufs=2, space="PSUM") as pap,
                tc.tile_pool(name="pn", bufs=4, space="PSUM") as pnp,
                tc.tile_pool(name="ot", bufs=4) as otp,
            ):
                for jj in range(mt):
                    xg = gp.tile([128, D], f16)
                    nc.gpsimd.indirect_dma_start(
                        out=xg[:], out_offset=None,
                        in_=emb[:],
                        in_offset=bass.IndirectOffsetOnAxis(
                            ap=tokens_sb[:, jj:jj + 1], axis=0),
                    )
                    xt_ps = pap.tile([128, D], f16, space="PSUM")
                    for k in range(DKT):
                        nc.tensor.transpose(
                            xt_ps[:, k * 128:(k + 1) * 128],
                            xg[:, k * 128:(k + 1) * 128],
                            ident[:],
                        )
                    xt = xtp.tile([128, D], f16)
                    nc.vector.tensor_copy(xt[:], xt_ps[:])
                    for n in range(NT):
                        ps = pnp.tile([128, 512], f32, space="PSUM")
                        for k in range(DKT):
                            nc.tensor.matmul(
                                ps[:],
                                lhsT=xt[:, k * 128:(k + 1) * 128],
                                rhs=W_sb[:, (k * NT + n) * 512:(k * NT + n + 1) * 512],
                                start=(k == 0), stop=(k == DKT - 1),
                            )
                        ot = otp.tile([128, 512], f16)
                        nc.vector.tensor_add(
                            ot[:], ps[:], bias_sb[:, n * 512:(n + 1) * 512])
                        nc.gpsimd.dma_start(
                            xp0_d[jj * CH:(jj + 1) * CH,
                                  BL * n:BL * n + BL, :],
                            ot[:])

            # ---------------- recurrence phase builder ----------------
            def recurrence(layer, xp_src, interleave=None):
                with (
                    nc.named_scope(f"rec{layer}"),
                    tc.tile_pool(name=f"st{layer}", bufs=2) as stp,
                    tc.tile_pool(name=f"xp{layer}", bufs=2) as xpp,
                    tc.tile_pool(name=f"hb{layer}", bufs=1) as hbp,
                    tc.tile_pool(name=f"pr{layer}", bufs=2, space="PSUM") as prp,
                ):
                    hT_sb = stp.tile([128, KT * 32], f16, tag="hT")
                    nc.gpsimd.memset(hT_sb[:], 0.0)
                    h_sb = hbp.tile([128, 512], f16)
                    # persistent double-buffered xp and PSUM tiles; memset
                    # once so the garbage lanes (16..31 of each quadrant)
                    # stay finite/zero forever.
                    xps = []
                    pss = []
                    for _ in range(2):
                        xp_t = xpp.tile([128, 512], f16)
                        nc.gpsimd.memset(xp_t[:], 0.0)
                        xps.append(xp_t)
                        ps = prp.tile([128, 512], f32, space="PSUM")
                        nc.vector.memset(ps[:], 0.0)
                        pss.append(ps)
                    for t in range(t_steps):
                        xp_t = xps[t % 2]
                        ps = pss[t % 2]
                        nc.gpsimd.dma_start(
                            xp_t[0:4 * BL, :], xp_src[t, :, :])
                        # round 0: inject xp into PSUM via identity-select
                        # matmuls (P_j[p,b] = [p == 16j+b]); rounds 1..16
                        # accumulate the recurrence on top. Col-tiled: group
                        # j streams W n-chunk j; the four 512-col streams
                        # run concurrently on the PE.
                        for j in range(NT):
                            nc.tensor.matmul(
                                ps[32 * j:32 * j + BL, :],
                                lhsT=ident[:, BL * j:BL * j + BL],
                                rhs=xp_t[:, :],
                                start=True, stop=False,
                                tile_position=(0, 32 * j),
                                skip_group_check=True,
                            )
                        for k in range(KT):
                            for j in range(NT):
                                nc.tensor.matmul(
                                    ps[32 * j:32 * j + BL, :],
                                    lhsT=hT_sb[:, k * 32:k * 32 + BL],
                                    rhs=W_sb[:, (k * NT + j) * 512:(k * NT + j + 1) * 512],
                                    start=False, stop=(k == KT - 1),
                                    tile_position=(0, 32 * j),
                                    skip_group_check=True,
                                )
                        hT_next = stp.tile([128, KT * 32], f16, tag="hT")
                        # tanh + single DVE 32x32 block-transpose (each block
                        # transposes in place within its own partition
                        # quadrant; W rows+cols are host-permuted so this
                        # produces the k-slot layout directly), split in
                        # column halves: the first half unlocks next step's
                        # k-rounds 0..7 early.
                        for hh in range(2):
                            cs = slice(hh * 256, hh * 256 + 256)
                            nc.scalar.activation(h_sb[:, cs], ps[:, cs], Tanh)
                            nc.vector.transpose(hT_next[:, cs], h_sb[:, cs])
                        if layer == 0:
                            nc.gpsimd.dma_start(y0T_d[t, :, :], hT_next[:])
                        else:
                            nc.gpsimd.dma_start(
                                h1_d[t * 128:(t + 1) * 128, :], h_sb[:])
                        hT_sb = hT_next
                        if interleave is not None:
                            interleave(t)

            # ------- Phase B: layer-0 recurrence + interleaved xp1 -------
            load_w(W_sb, w_hh0, KT)
            W2_sb = wp.tile([128, KT * H], f16)
            load_w(W2_sb, w_ih1, KT)
            bias2_sb = st.tile([128, H], f32)
            load_bias(bias2_sb, b1)
            with (
                tc.tile_pool(name="lh", bufs=2) as lhp,
                tc.tile_pool(name="pc", bufs=2, space="PSUM") as pcp,
                tc.tile_pool(name="oc", bufs=4) as ocp,
            ):
                lh_tiles = {}

                def load_chunk(c):
                    lh = lhp.tile([128, H], f16)
                    lh_tiles[c] = lh
                    for k in range(KT):
                        nc.gpsimd.dma_start(
                            lh[:, k * 128:(k + 1) * 128]
                            .rearrange("p (t c2) -> p t c2", t=CH),
                            y0T_d[c * CH:(c + 1) * CH, :, k * 32:k * 32 + BL]
                            .rearrange("t p c2 -> p t c2"),
                        )

                def emit_chunk(c):
                    # xp1 chunk c (steps 8c..8c+7) on the otherwise-idle PE
                    # gaps of the recurrence; full-width M=128 matmuls. The
                    # lh load is prefetched several steps earlier so the
                    # slow strided DMA never stalls the in-order PE queue.
                    lh = lh_tiles.pop(c)
                    for n in range(NT):
                        ps = pcp.tile([128, 512], f32, space="PSUM")
                        for k in range(KT):
                            nc.tensor.matmul(
                                ps[:],
                                lhsT=lh[:, k * 128:(k + 1) * 128],
                                rhs=W2_sb[:, (k * NT + n) * 512:(k * NT + n + 1) * 512],
                                start=(k == 0), stop=(k == KT - 1),
                            )
                        oc = ocp.tile([128, 512], f16)
                        nc.vector.tensor_add(
                            oc[:], ps[:], bias2_sb[:, n * 512:(n + 1) * 512])
                        nc.gpsimd.dma_start(
                            xp1_d[c * CH:(c + 1) * CH,
                                  BL * n:BL * n + BL, :],
                            oc[:])

                def interleave(t):
                    if t >= CH + 3 and (t - (CH + 3)) % CH == 0:
                        load_chunk((t - (CH + 3)) // CH)
                    if t >= 2 * CH - 1 and (t - (2 * CH - 1)) % CH == 0:
                        emit_chunk((t - (2 * CH - 1)) // CH)

                recurrence(0, xp0_d, interleave=interleave)
                with nc.named_scope("phaseC"):
                    load_chunk(nchunk - 1)
                    emit_chunk(nchunk - 1)

            # ---------------- Phase D: layer-1 recurrence ----------------
            load_w(W_sb, w_hh1, KT)
            recurrence(1, xp1_d)

            # final capture: out row 16j+b = h1[b, chunk j perm cols] at t=len_b-1
            with tc.tile_pool(name="cap", bufs=1) as cp:
                ci = cp.tile([4 * BL, 1], i32)
                nc.gpsimd.dma_start(ci[:], cap_idx[:, :])
                og = cp.tile([4 * BL, 512], f16)
                nc.gpsimd.indirect_dma_start(
                    out=og[:], out_offset=None,
                    in_=h1_d[:],
                    in_offset=bass.IndirectOffsetOnAxis(ap=ci[:, :1], axis=0),
                )
                nc.gpsimd.dma_start(out_h[:, :], og[:])

    nc.finalize()
    return nc


def _install_ntff_hook():
    """The trimmed agent image lacks antenv.axon_hooks — provide the tiny
    get/set registry and install the ctypes NTFF hook so trace=True works."""
    import types

    if "antenv.axon_hooks" in sys.modules:
        return
    m = types.ModuleType("antenv.axon_hooks")
    _hook = [None]
    m.set_axon_ntff_profile_hook = lambda h: _hook.__setitem__(0, h)
    m.get_axon_ntff_profile_hook = lambda: _hook[0]
    sys.modules["antenv.axon_hooks"] = m
    import antenv
    antenv.axon_hooks = m
    try:
        from trn_agent_boot.trn_boot import _ntff_profile_via_ctypes
        hook = _ntff_profile_via_ctypes("/opt/axon/libaxon_pjrt.so")
        if hook is not None:
            m.set_axon_ntff_profile_hook(hook)
        import concourse.bass_utils as bu
        bu.upload_artifacts = lambda d: str(d)
    except Exception:
        pass


def _permute_cols(w):
    """Swap the (mm, q) 32-col block fields within each 512-col chunk of the
    last dim: position 128q+32mm+v of a chunk holds natural col 128mm+32q+v.
    Involution. Makes each DVE transpose source a contiguous [32,128] slice."""
    shp = w.shape
    wr = w.reshape(-1, H // 512, 4, 4, 32)
    return np.ascontiguousarray(
        wr.transpose(0, 1, 3, 2, 4).reshape(shp))


_ROW_PERM = None


def _row_perm():
    """Contraction-row order matching the hT slot layout produced by the
    in-quadrant DVE block transpose: W_sb slot ss, partition 32j+v holds
    natural h-dim 512j + 128*(ss%4) + 32*(ss//4) + v."""
    global _ROW_PERM
    if _ROW_PERM is None:
        idx = np.empty(H, np.int64)
        for ss in range(KT):
            for j in range(4):
                v = np.arange(32)
                idx[ss * 128 + 32 * j + v] = (
                    512 * j + 128 * (ss % 4) + 32 * (ss // 4) + v)
        _ROW_PERM = idx
    return _ROW_PERM


def _make_in_maps(tokens, lengths, emb, W_ih0, W_hh0, b0, W_ih1, W_hh1, b1, ts):
    rp = _row_perm()
    W_ih0 = _permute_cols(W_ih0).astype(np.float16)
    W_hh0 = _permute_cols(W_hh0[rp]).astype(np.float16)
    W_ih1 = _permute_cols(W_ih1[rp]).astype(np.float16)
    W_hh1 = _permute_cols(W_hh1[rp]).astype(np.float16)
    b0 = _permute_cols(b0)
    b1 = _permute_cols(b1)
    emb16 = np.ascontiguousarray(emb.astype(np.float16))
    in_maps = []
    for c in range(NC):
        tok_c = tokens[c * BL:(c + 1) * BL, :ts]          # [16, ts]
        flat = tok_c.T.reshape(-1)                        # t-major rows
        tokT = np.ascontiguousarray(flat.reshape(-1, 128).T)  # [128, mt]
        len_c = np.minimum(lengths[c * BL:(c + 1) * BL].astype(np.int64), ts)
        r = np.arange(4 * BL)
        cap = ((len_c[r % BL] - 1) * 128 + 32 * (r // BL)
               + (r % BL)).astype(np.int32)[:, None]
        in_maps.append({
            "tokT": tokT,
            "cap_idx": np.ascontiguousarray(cap),
            "emb": emb16,
            "w_ih0": W_ih0, "w_hh0": W_hh0, "b0": b0,
            "w_ih1": W_ih1, "w_hh1": W_hh1, "b1": b1,
        })
    return in_maps


def _assemble(out_h_core):
    # out row 16j+b, col 128q+32mm+v -> h[b, 512j+128mm+32q+v]
    return (np.asarray(out_h_core).astype(np.float32)
            .reshape(4, BL, 4, 4, 32)
            .transpose(1, 0, 3, 2, 4).reshape(BL, H))


def kernel(tokens, lengths, emb, W_ih0, W_hh0, b0, W_ih1, W_hh1, b1,
           _t_steps=T, _trace=False):
    from concourse.bass_utils import run_bass_kernel_spmd

    if _trace:
        _install_ntff_hook()

    tokens = np.asarray(tokens).astype(np.int32)
    lengths = np.asarray(lengths).astype(np.int32)
    emb = np.ascontiguousarray(np.asarray(emb, dtype=np.float32))
    W_ih0 = np.ascontiguousarray(np.asarray(W_ih0, dtype=np.float32))
    W_hh0 = np.ascontiguousarray(np.asarray(W_hh0, dtype=np.float32))
    W_ih1 = np.ascontiguousarray(np.asarray(W_ih1, dtype=np.float32))
    W_hh1 = np.ascontiguousarray(np.asarray(W_hh1, dtype=np.float32))
    b0 = np.ascontiguousarray(np.asarray(b0, dtype=np.float32).reshape(1, H))
    b1 = np.ascontiguousarray(np.asarray(b1, dtype=np.float32).reshape(1, H))

    ts = _t_steps
    if ts not in _CACHE:
        _CACHE[ts] = _build(ts)
    nc = _CACHE[ts]

    in_maps = _make_in_maps(tokens, lengths, emb, W_ih0, W_hh0, b0,
                            W_ih1, W_hh1, b1, ts)

    res = run_bass_kernel_spmd(nc, in_maps, list(range(NC)), trace=_trace)
    STATS["exec_time_ns"] = res.exec_time_ns
    STATS["mean_exec_time_ns"] = res.mean_exec_time_ns
    STATS["scope_times"] = res.per_core_scope_times
    out = np.concatenate(
        [_assemble(res.results[c]["out_h"]) for c in range(NC)], axis=0)
    return out.astype(np.float32)


# revision 6
# speedup vs baseline: 1.1370x; 1.1370x over previous
"""Trainium2 Bass kernel for a 2-layer masked (ragged) Elman RNN — v2.

Problem: tokens [128,512] -> emb lookup [B,T,1024] -> RNN(1024->2048) ->
RNN(2048->2048) -> final hidden of layer 1, with per-sequence lengths
freezing the hidden state at t >= len (packed-sequence semantics).

Strategy (8 NeuronCores, data-parallel over batch, 16 seqs/core):
  Phase A: embedding gather + bulk input projection xp0 = X@W_ih0 + b0
  Phase B: layer-0 recurrence (512 serial steps)
  Phase C: bulk xp1 = y0 @ W_ih1 + b1 in chunks of 8 timesteps
  Phase D: layer-1 recurrence, storing h1 per step to DRAM; final
           per-sequence capture via indirect gather at t = len-1.

v2 changes vs baseline:
- The recurrence matmuls have M=16 (batch per core), using only 16 of the
  PE's 128 columns. They are now issued to the four 32-column PE tile
  groups (tile_position=(0,32j)): group j computes output n-chunk j with
  the full k accumulation, so four 512-col W streams run concurrently.
  Col-tiling requires 16-bit operands (fp32r matmuls fail the ISA check),
  so the whole matmul datapath is fp16 (same 10-bit mantissa as TF32;
  PSUM accumulation stays fp32).
- Pre-activations land in one PSUM bank in a "grouped" layout: partition
  32j+b holds pre[b, perm(512j+n')]. W columns are host-permuted (swap of
  the two 32-col block fields within each 512 chunk) so the per-step
  hidden-state transposition becomes 16 contiguous [32,128] DVE 32x32
  stream-transposes — entirely off the PE.
- xp tensors are stored in DRAM pre-swizzled to the grouped layout.

The recurrences run unmasked: for t < len the unmasked state equals the
reference's frozen-state values, and the capture row only reads t=len-1.
"""

import sys

sys.path.insert(0, "/opt/trn_rl_repo")

import numpy as np

B, T, V, D, H = 128, 512, 32000, 1024, 2048
NC = 8
BL = B // NC          # 16 sequences per core
KT = H // 128         # 16 k-tiles of the hidden dim
NT = H // 512         # 4 n-tiles (PSUM bank width)
DKT = D // 128        # 8 k-tiles of the embedding dim
CH = 128 // BL        # 8 timesteps per phase-A/C GEMM tile

STATS = {}
_CACHE = {}


def _build(t_steps):
    import concourse.bass as bass
    import concourse.mybir as mybir
    import concourse.tile as tile
    from concourse import bacc
    from concourse.masks import make_identity

    f32 = mybir.dt.float32
    f16 = mybir.dt.float16
    i32 = mybir.dt.int32
    Tanh = mybir.ActivationFunctionType.Tanh

    mt = (t_steps * BL) // 128   # phase-A tile count
    nchunk = t_steps // CH       # phase-C chunk count

    nc = bacc.Bacc("TRN2", target_bir_lowering=False, debug=False, num_devices=NC)

    tokT = nc.dram_tensor("tokT", [128, mt], i32, kind="ExternalInput")
    cap_idx = nc.dram_tensor("cap_idx", [4 * BL, 1], i32, kind="ExternalInput")
    emb = nc.dram_tensor("emb", [V, D], f16, kind="ExternalInput")
    w_ih0 = nc.dram_tensor("w_ih0", [D, H], f16, kind="ExternalInput")
    w_hh0 = nc.dram_tensor("w_hh0", [H, H], f16, kind="ExternalInput")
    b0 = nc.dram_tensor("b0", [1, H], f32, kind="ExternalInput")
    w_ih1 = nc.dram_tensor("w_ih1", [H, H], f16, kind="ExternalInput")
    w_hh1 = nc.dram_tensor("w_hh1", [H, H], f16, kind="ExternalInput")
    b1 = nc.dram_tensor("b1", [1, H], f32, kind="ExternalInput")
    out_h = nc.dram_tensor("out_h", [4 * BL, 512], f16, kind="ExternalOutput")

    # compact grouped layout: [t, 16*j + b, n'] = value[b, perm(512*j + n')]
    xp0_d = nc.dram_tensor("xp0_d", [t_steps, 4 * BL, 512], f16)
    xp1_d = nc.dram_tensor("xp1_d", [t_steps, 4 * BL, 512], f16)
    # per-step transposed state, compacted slots: [t, p, 16*k + b]
    y0T_d = nc.dram_tensor("y0T_d", [t_steps, 128, 256], f16)
    # layer-1 h per step, grouped rows (32*j + b valid for b < 16)
    h1_d = nc.dram_tensor("h1_d", [t_steps * 128, 512], f16)

    def load_w(W_sb, wsrc, ktiles):
        # W_sb col block (k*NT+n)*512 holds wsrc[k*128:(k+1)*128, n*512:(n+1)*512]
        for k in range(ktiles):
            nc.gpsimd.dma_start(
                W_sb[:, k * H:(k + 1) * H],
                wsrc[k * 128:(k + 1) * 128, :],
            )

    def load_bias(bias_sb, bsrc):
        nc.gpsimd.dma_start(bias_sb[0:1, :], bsrc[0:1, :])
        nc.gpsimd.partition_broadcast(bias_sb[:], bias_sb[0:1, :])

    with tile.TileContext(nc) as tc:
        with (
            tc.tile_pool(name="wpool", bufs=1) as wp,
            tc.tile_pool(name="state", bufs=1) as st,
        ):
            W_sb = wp.tile([128, KT * H], f16)       # 64KB/partition
            ident = st.tile([128, 128], f16)
            make_identity(nc, ident[:])
            bias_sb = st.tile([128, H], f32)
            tokens_sb = st.tile([128, mt], i32)
            nc.gpsimd.dma_start(tokens_sb[:], tokT[:, :])

            # ---------------- Phase A: embed + xp0 ----------------
            load_w(W_sb, w_ih0, DKT)
            load_bias(bias_sb, b0)
            with (
                nc.named_scope("phaseA"),
                tc.tile_pool(name="ga", bufs=3) as gp,
                tc.tile_pool(name="xt", bufs=2) as xtp,
                tc.tile_pool(name="pa", bufs=2, space="PSUM") as pap,
                tc.tile_pool(name="pn", bufs=4, space="PSUM") as pnp,
                tc.tile_pool(name="ot", bufs=4) as otp,
            ):
                for jj in range(mt):
                    xg = gp.tile([128, D], f16)
                    nc.gpsimd.indirect_dma_start(
                        out=xg[:], out_offset=None,
                        in_=emb[:],
                        in_offset=bass.IndirectOffsetOnAxis(
                            ap=tokens_sb[:, jj:jj + 1], axis=0),
                    )
                    xt_ps = pap.tile([128, D], f16, space="PSUM")
                    for k in range(DKT):
                        nc.tensor.transpose(
                            xt_ps[:, k * 128:(k + 1) * 128],
                            xg[:, k * 128:(k + 1) * 128],
                            ident[:],
                        )
                    xt = xtp.tile([128, D], f16)
                    nc.vector.tensor_copy(xt[:], xt_ps[:])
                    for n in range(NT):
                        ps = pnp.tile([128, 512], f32, space="PSUM")
                        for k in range(DKT):
                            nc.tensor.matmul(
                                ps[:],
                                lhsT=xt[:, k * 128:(k + 1) * 128],
                                rhs=W_sb[:, (k * NT + n) * 512:(k * NT + n + 1) * 512],
                                start=(k == 0), stop=(k == DKT - 1),
                            )
                        ot = otp.tile([128, 512], f16)
                        nc.vector.tensor_add(
                            ot[:], ps[:], bias_sb[:, n * 512:(n + 1) * 512])
                        nc.gpsimd.dma_start(
                            xp0_d[jj * CH:(jj + 1) * CH,
                                  BL * n:BL * n + BL, :],
                            ot[:])

            # ---------------- recurrence phase builder ----------------
            def recurrence(layer, xp_src, interleave=None):
                with (
                    nc.named_scope(f"rec{layer}"),
                    tc.tile_pool(name=f"st{layer}", bufs=2) as stp,
                    tc.tile_pool(name=f"xp{layer}", bufs=2) as xpp,
                    tc.tile_pool(name=f"hb{layer}", bufs=1) as hbp,
                    tc.tile_pool(name=f"pr{layer}", bufs=2, space="PSUM") as prp,
                ):
                    hT_sb = stp.tile([128, KT * 32], f16, tag="hT")
                    nc.gpsimd.memset(hT_sb[:], 0.0)
                    h_sb = hbp.tile([128, 512], f16)
                    # persistent double-buffered xp and PSUM tiles; memset
                    # once so the garbage lanes (16..31 of each quadrant)
                    # stay finite/zero forever.
                    xps = []
                    pss = []
                    for _ in range(2):
                        xp_t = xpp.tile([128, 512], f16)
                        nc.gpsimd.memset(xp_t[:], 0.0)
                        xps.append(xp_t)
                        ps = prp.tile([128, 512], f32, space="PSUM")
                        nc.vector.memset(ps[:], 0.0)
                        pss.append(ps)
                    for t in range(t_steps):
                        xp_t = xps[t % 2]
                        ps = pss[t % 2]
                        nc.gpsimd.dma_start(
                            xp_t[0:4 * BL, :], xp_src[t, :, :])
                        # round 0: inject xp into PSUM via identity-select
                        # matmuls (P_j[p,b] = [p == 16j+b]); rounds 1..16
                        # accumulate the recurrence on top. Col-tiled: group
                        # j streams W n-chunk j; the four 512-col streams
                        # run concurrently on the PE.
                        for j in range(NT):
                            nc.tensor.matmul(
                                ps[32 * j:32 * j + BL, :],
                                lhsT=ident[:, BL * j:BL * j + BL],
                                rhs=xp_t[:, :],
                                start=True, stop=False,
                                tile_position=(0, 32 * j),
                                skip_group_check=True,
                            )
                        for k in range(KT):
                            for j in range(NT):
                                nc.tensor.matmul(
                                    ps[32 * j:32 * j + BL, :],
                                    lhsT=hT_sb[:, k * 32:k * 32 + BL],
                                    rhs=W_sb[:, (k * NT + j) * 512:(k * NT + j + 1) * 512],
                                    start=False, stop=(k == KT - 1),
                                    tile_position=(0, 32 * j),
                                    skip_group_check=True,
                                )
                        hT_next = stp.tile([128, KT * 32], f16, tag="hT")
                        # tanh + single DVE 32x32 block-transpose (each block
                        # transposes in place within its own partition
                        # quadrant; W rows+cols are host-permuted so this
                        # produces the k-slot layout directly), split in
                        # column halves: the first half unlocks next step's
                        # k-rounds 0..7 early.
                        for hh in range(2):
                            cs = slice(hh * 256, hh * 256 + 256)
                            nc.scalar.activation(h_sb[:, cs], ps[:, cs], Tanh)
                            nc.vector.transpose(hT_next[:, cs], h_sb[:, cs])
                        if layer == 0:
                            nc.gpsimd.dma_start(
                                y0T_d[t, :, :],
                                hT_next[:, :]
                                .rearrange("p (k b2) -> p k b2", k=KT)
                                [:, :, 0:BL])
                        else:
                            nc.gpsimd.dma_start(
                                h1_d[t * 128:(t + 1) * 128, :], h_sb[:])
                        hT_sb = hT_next
                        if interleave is not None:
                            interleave(t)

            # ------- Phase B: layer-0 recurrence + interleaved xp1 -------
            load_w(W_sb, w_hh0, KT)
            W2_sb = wp.tile([128, KT * H], f16)
            load_w(W2_sb, w_ih1, KT)
            bias2_sb = st.tile([128, H], f32)
            load_bias(bias2_sb, b1)
            with (
                tc.tile_pool(name="lh", bufs=2) as lhp,
                tc.tile_pool(name="pc", bufs=2, space="PSUM") as pcp,
                tc.tile_pool(name="oc", bufs=4) as ocp,
            ):
                lh_tiles = {}

                def load_chunk(c):
                    # single contiguous [p, t, 256] DMA (512B runs) into a
                    # t-major staging tile, then one DVE re-tile copy into
                    # the k-major layout the stationary matmul operand needs.
                    lh_raw = lhp.tile([128, H], f16, tag="lhr")
                    nc.gpsimd.dma_start(
                        lh_raw[:, :],
                        y0T_d[c * CH:(c + 1) * CH, :, :]
                        .rearrange("t p c2 -> p t c2"),
                    )
                    lh = lhp.tile([128, H], f16, tag="lh")
                    lh_tiles[c] = lh
                    nc.vector.tensor_copy(
                        lh[:, :].rearrange("p (kk t b2) -> p kk t b2",
                                           kk=KT, t=CH),
                        lh_raw[:, :].rearrange("p (t kk b2) -> p kk t b2",
                                               t=CH, kk=KT),
                    )

                def emit_chunk(c):
                    # xp1 chunk c (steps 8c..8c+7) on the otherwise-idle PE
                    # gaps of the recurrence; full-width M=128 matmuls. The
                    # lh load is prefetched several steps earlier so the
                    # slow strided DMA never stalls the in-order PE queue.
                    lh = lh_tiles.pop(c)
                    for n in range(NT):
                        ps = pcp.tile([128, 512], f32, space="PSUM")
                        for k in range(KT):
                            nc.tensor.matmul(
                                ps[:],
                                lhsT=lh[:, k * 128:(k + 1) * 128],
                                rhs=W2_sb[:, (k * NT + n) * 512:(k * NT + n + 1) * 512],
                                start=(k == 0), stop=(k == KT - 1),
                            )
                        oc = ocp.tile([128, 512], f16)
                        nc.vector.tensor_add(
                            oc[:], ps[:], bias2_sb[:, n * 512:(n + 1) * 512])
                        nc.gpsimd.dma_start(
                            xp1_d[c * CH:(c + 1) * CH,
                                  BL * n:BL * n + BL, :],
                            oc[:])

                def interleave(t):
                    if t >= CH + 3 and (t - (CH + 3)) % CH == 0:
                        load_chunk((t - (CH + 3)) // CH)
                    if t >= 2 * CH - 1 and (t - (2 * CH - 1)) % CH == 0:
                        emit_chunk((t - (2 * CH - 1)) // CH)

                recurrence(0, xp0_d, interleave=interleave)
                with nc.named_scope("phaseC"):
                    load_chunk(nchunk - 1)
                    emit_chunk(nchunk - 1)

            # ---------------- Phase D: layer-1 recurrence ----------------
            load_w(W_sb, w_hh1, KT)
            recurrence(1, xp1_d)

            # final capture: out row 16j+b = h1[b, chunk j perm cols] at t=len_b-1
            with tc.tile_pool(name="cap", bufs=1) as cp:
                ci = cp.tile([4 * BL, 1], i32)
                nc.gpsimd.dma_start(ci[:], cap_idx[:, :])
                og = cp.tile([4 * BL, 512], f16)
                nc.gpsimd.indirect_dma_start(
                    out=og[:], out_offset=None,
                    in_=h1_d[:],
                    in_offset=bass.IndirectOffsetOnAxis(ap=ci[:, :1], axis=0),
                )
                nc.gpsimd.dma_start(out_h[:, :], og[:])

    nc.finalize()
    return nc


def _install_ntff_hook():
    """The trimmed agent image lacks antenv.axon_hooks — provide the tiny
    get/set registry and install the ctypes NTFF hook so trace=True works."""
    import types

    if "antenv.axon_hooks" in sys.modules:
        return
    m = types.ModuleType("antenv.axon_hooks")
    _hook = [None]
    m.set_axon_ntff_profile_hook = lambda h: _hook.__setitem__(0, h)
    m.get_axon_ntff_profile_hook = lambda: _hook[0]
    sys.modules["antenv.axon_hooks"] = m
    import antenv
    antenv.axon_hooks = m
    try:
        from trn_agent_boot.trn_boot import _ntff_profile_via_ctypes
        hook = _ntff_profile_via_ctypes("/opt/axon/libaxon_pjrt.so")
        if hook is not None:
            m.set_axon_ntff_profile_hook(hook)
        import concourse.bass_utils as bu
        bu.upload_artifacts = lambda d: str(d)
    except Exception:
        pass


def _permute_cols(w):
    """Swap the (mm, q) 32-col block fields within each 512-col chunk of the
    last dim: position 128q+32mm+v of a chunk holds natural col 128mm+32q+v.
    Involution. Makes each DVE transpose source a contiguous [32,128] slice."""
    shp = w.shape
    wr = w.reshape(-1, H // 512, 4, 4, 32)
    return np.ascontiguousarray(
        wr.transpose(0, 1, 3, 2, 4).reshape(shp))


_ROW_PERM = None


def _row_perm():
    """Contraction-row order matching the hT slot layout produced by the
    in-quadrant DVE block transpose: W_sb slot ss, partition 32j+v holds
    natural h-dim 512j + 128*(ss%4) + 32*(ss//4) + v."""
    global _ROW_PERM
    if _ROW_PERM is None:
        idx = np.empty(H, np.int64)
        for ss in range(KT):
            for j in range(4):
                v = np.arange(32)
                idx[ss * 128 + 32 * j + v] = (
                    512 * j + 128 * (ss % 4) + 32 * (ss // 4) + v)
        _ROW_PERM = idx
    return _ROW_PERM


def _make_in_maps(tokens, lengths, emb, W_ih0, W_hh0, b0, W_ih1, W_hh1, b1, ts):
    rp = _row_perm()
    W_ih0 = _permute_cols(W_ih0).astype(np.float16)
    W_hh0 = _permute_cols(W_hh0[rp]).astype(np.float16)
    W_ih1 = _permute_cols(W_ih1[rp]).astype(np.float16)
    W_hh1 = _permute_cols(W_hh1[rp]).astype(np.float16)
    b0 = _permute_cols(b0)
    b1 = _permute_cols(b1)
    emb16 = np.ascontiguousarray(emb.astype(np.float16))
    in_maps = []
    for c in range(NC):
        tok_c = tokens[c * BL:(c + 1) * BL, :ts]          # [16, ts]
        flat = tok_c.T.reshape(-1)                        # t-major rows
        tokT = np.ascontiguousarray(flat.reshape(-1, 128).T)  # [128, mt]
        len_c = np.minimum(lengths[c * BL:(c + 1) * BL].astype(np.int64), ts)
        r = np.arange(4 * BL)
        cap = ((len_c[r % BL] - 1) * 128 + 32 * (r // BL)
               + (r % BL)).astype(np.int32)[:, None]
        in_maps.append({
            "tokT": tokT,
            "cap_idx": np.ascontiguousarray(cap),
            "emb": emb16,
            "w_ih0": W_ih0, "w_hh0": W_hh0, "b0": b0,
            "w_ih1": W_ih1, "w_hh1": W_hh1, "b1": b1,
        })
    return in_maps


def _assemble(out_h_core):
    # out row 16j+b, col 128q+32mm+v -> h[b, 512j+128mm+32q+v]
    return (np.asarray(out_h_core).astype(np.float32)
            .reshape(4, BL, 4, 4, 32)
            .transpose(1, 0, 3, 2, 4).reshape(BL, H))


def kernel(tokens, lengths, emb, W_ih0, W_hh0, b0, W_ih1, W_hh1, b1,
           _t_steps=T, _trace=False):
    from concourse.bass_utils import run_bass_kernel_spmd

    if _trace:
        _install_ntff_hook()

    tokens = np.asarray(tokens).astype(np.int32)
    lengths = np.asarray(lengths).astype(np.int32)
    emb = np.ascontiguousarray(np.asarray(emb, dtype=np.float32))
    W_ih0 = np.ascontiguousarray(np.asarray(W_ih0, dtype=np.float32))
    W_hh0 = np.ascontiguousarray(np.asarray(W_hh0, dtype=np.float32))
    W_ih1 = np.ascontiguousarray(np.asarray(W_ih1, dtype=np.float32))
    W_hh1 = np.ascontiguousarray(np.asarray(W_hh1, dtype=np.float32))
    b0 = np.ascontiguousarray(np.asarray(b0, dtype=np.float32).reshape(1, H))
    b1 = np.ascontiguousarray(np.asarray(b1, dtype=np.float32).reshape(1, H))

    ts = _t_steps
    if ts not in _CACHE:
        _CACHE[ts] = _build(ts)
    nc = _CACHE[ts]

    in_maps = _make_in_maps(tokens, lengths, emb, W_ih0, W_hh0, b0,
                            W_ih1, W_hh1, b1, ts)

    res = run_bass_kernel_spmd(nc, in_maps, list(range(NC)), trace=_trace)
    STATS["exec_time_ns"] = res.exec_time_ns
    STATS["mean_exec_time_ns"] = res.mean_exec_time_ns
    STATS["scope_times"] = res.per_core_scope_times
    out = np.concatenate(
        [_assemble(res.results[c]["out_h"]) for c in range(NC)], axis=0)
    return out.astype(np.float32)


# revision 7
# speedup vs baseline: 1.1606x; 1.0207x over previous
"""Trainium2 Bass kernel for a 2-layer masked (ragged) Elman RNN — v2.

Problem: tokens [128,512] -> emb lookup [B,T,1024] -> RNN(1024->2048) ->
RNN(2048->2048) -> final hidden of layer 1, with per-sequence lengths
freezing the hidden state at t >= len (packed-sequence semantics).

Strategy (8 NeuronCores, data-parallel over batch, 16 seqs/core):
  Phase A: embedding gather + bulk input projection xp0 = X@W_ih0 + b0
  Phase B: layer-0 recurrence (512 serial steps)
  Phase C: bulk xp1 = y0 @ W_ih1 + b1 in chunks of 8 timesteps
  Phase D: layer-1 recurrence, storing h1 per step to DRAM; final
           per-sequence capture via indirect gather at t = len-1.

v2 changes vs baseline:
- The recurrence matmuls have M=16 (batch per core), using only 16 of the
  PE's 128 columns. They are now issued to the four 32-column PE tile
  groups (tile_position=(0,32j)): group j computes output n-chunk j with
  the full k accumulation, so four 512-col W streams run concurrently.
  Col-tiling requires 16-bit operands (fp32r matmuls fail the ISA check),
  so the whole matmul datapath is fp16 (same 10-bit mantissa as TF32;
  PSUM accumulation stays fp32).
- Pre-activations land in one PSUM bank in a "grouped" layout: partition
  32j+b holds pre[b, perm(512j+n')]. W columns are host-permuted (swap of
  the two 32-col block fields within each 512 chunk) so the per-step
  hidden-state transposition becomes 16 contiguous [32,128] DVE 32x32
  stream-transposes — entirely off the PE.
- xp tensors are stored in DRAM pre-swizzled to the grouped layout.

The recurrences run unmasked: for t < len the unmasked state equals the
reference's frozen-state values, and the capture row only reads t=len-1.
"""

import sys

sys.path.insert(0, "/opt/trn_rl_repo")

import numpy as np

B, T, V, D, H = 128, 512, 32000, 1024, 2048
NC = 8
BL = B // NC          # 16 sequences per core
KT = H // 128         # 16 k-tiles of the hidden dim
NT = H // 512         # 4 n-tiles (PSUM bank width)
DKT = D // 128        # 8 k-tiles of the embedding dim
CH = 128 // BL        # 8 timesteps per phase-A/C GEMM tile

STATS = {}
_CACHE = {}


def _build(t_steps):
    import concourse.bass as bass
    import concourse.mybir as mybir
    import concourse.tile as tile
    from concourse import bacc
    from concourse.masks import make_identity

    f32 = mybir.dt.float32
    f16 = mybir.dt.float16
    i32 = mybir.dt.int32
    Tanh = mybir.ActivationFunctionType.Tanh

    mt = (t_steps * BL) // 128   # phase-A tile count
    nchunk = t_steps // CH       # phase-C chunk count

    nc = bacc.Bacc("TRN2", target_bir_lowering=False, debug=False, num_devices=NC)

    tokT = nc.dram_tensor("tokT", [128, mt], i32, kind="ExternalInput")
    cap_idx = nc.dram_tensor("cap_idx", [4 * BL, 1], i32, kind="ExternalInput")
    emb = nc.dram_tensor("emb", [V, D], f16, kind="ExternalInput")
    w_ih0 = nc.dram_tensor("w_ih0", [D, H], f16, kind="ExternalInput")
    w_hh0 = nc.dram_tensor("w_hh0", [H, H], f16, kind="ExternalInput")
    b0 = nc.dram_tensor("b0", [1, H], f32, kind="ExternalInput")
    w_ih1 = nc.dram_tensor("w_ih1", [H, H], f16, kind="ExternalInput")
    w_hh1 = nc.dram_tensor("w_hh1", [H, H], f16, kind="ExternalInput")
    b1 = nc.dram_tensor("b1", [1, H], f32, kind="ExternalInput")
    out_h = nc.dram_tensor("out_h", [4 * BL, 512], f16, kind="ExternalOutput")

    # compact grouped layout: [t, 16*j + b, n'] = value[b, perm(512*j + n')]
    xp0_d = nc.dram_tensor("xp0_d", [t_steps, 4 * BL, 512], f16)
    xp1_d = nc.dram_tensor("xp1_d", [t_steps, 4 * BL, 512], f16)
    # per-step transposed state, compacted slots: [t, p, 16*k + b]
    y0T_d = nc.dram_tensor("y0T_d", [t_steps, 128, 256], f16)
    # layer-1 h per step, grouped rows (32*j + b valid for b < 16)
    h1_d = nc.dram_tensor("h1_d", [t_steps * 128, 512], f16)

    def load_w(W_sb, wsrc, ktiles):
        # W_sb col block (k*NT+n)*512 holds wsrc[k*128:(k+1)*128, n*512:(n+1)*512]
        for k in range(ktiles):
            nc.gpsimd.dma_start(
                W_sb[:, k * H:(k + 1) * H],
                wsrc[k * 128:(k + 1) * 128, :],
            )

    def load_bias(bias_sb, bsrc):
        nc.gpsimd.dma_start(bias_sb[0:1, :], bsrc[0:1, :])
        nc.gpsimd.partition_broadcast(bias_sb[:], bias_sb[0:1, :])

    with tile.TileContext(nc) as tc:
        with (
            tc.tile_pool(name="wpool", bufs=1) as wp,
            tc.tile_pool(name="state", bufs=1) as st,
        ):
            W_sb = wp.tile([128, KT * H], f16)       # 64KB/partition
            ident = st.tile([128, 128], f16)
            make_identity(nc, ident[:])
            bias_sb = st.tile([128, H], f32)
            tokens_sb = st.tile([128, mt], i32)
            nc.gpsimd.dma_start(tokens_sb[:], tokT[:, :])

            # ---------------- Phase A: embed + xp0 ----------------
            load_w(W_sb, w_ih0, DKT)
            load_bias(bias_sb, b0)
            with (
                nc.named_scope("phaseA"),
                tc.tile_pool(name="ga", bufs=3) as gp,
                tc.tile_pool(name="xt", bufs=2) as xtp,
                tc.tile_pool(name="pa", bufs=2, space="PSUM") as pap,
                tc.tile_pool(name="pn", bufs=4, space="PSUM") as pnp,
                tc.tile_pool(name="ot", bufs=4) as otp,
            ):
                for jj in range(mt):
                    xg = gp.tile([128, D], f16)
                    nc.gpsimd.indirect_dma_start(
                        out=xg[:], out_offset=None,
                        in_=emb[:],
                        in_offset=bass.IndirectOffsetOnAxis(
                            ap=tokens_sb[:, jj:jj + 1], axis=0),
                    )
                    xt_ps = pap.tile([128, D], f16, space="PSUM")
                    for k in range(DKT):
                        nc.tensor.transpose(
                            xt_ps[:, k * 128:(k + 1) * 128],
                            xg[:, k * 128:(k + 1) * 128],
                            ident[:],
                        )
                    xt = xtp.tile([128, D], f16)
                    nc.vector.tensor_copy(xt[:], xt_ps[:])
                    for n in range(NT):
                        ps = pnp.tile([128, 512], f32, space="PSUM")
                        for k in range(DKT):
                            nc.tensor.matmul(
                                ps[:],
                                lhsT=xt[:, k * 128:(k + 1) * 128],
                                rhs=W_sb[:, (k * NT + n) * 512:(k * NT + n + 1) * 512],
                                start=(k == 0), stop=(k == DKT - 1),
                            )
                        ot = otp.tile([128, 512], f16)
                        nc.vector.tensor_add(
                            ot[:], ps[:], bias_sb[:, n * 512:(n + 1) * 512])
                        nc.gpsimd.dma_start(
                            xp0_d[jj * CH:(jj + 1) * CH,
                                  BL * n:BL * n + BL, :],
                            ot[:])

            # ---------------- recurrence phase builder ----------------
            def recurrence(layer, xp_src, interleave=None):
                with (
                    nc.named_scope(f"rec{layer}"),
                    tc.tile_pool(name=f"st{layer}", bufs=2) as stp,
                    tc.tile_pool(name=f"xp{layer}", bufs=2) as xpp,
                    tc.tile_pool(name=f"hb{layer}", bufs=1) as hbp,
                    tc.tile_pool(name=f"pr{layer}", bufs=2, space="PSUM") as prp,
                ):
                    hT_sb = stp.tile([128, KT * 32], f16, tag="hT")
                    nc.gpsimd.memset(hT_sb[:], 0.0)
                    h_sb = hbp.tile([128, 512], f16)
                    # persistent double-buffered xp and PSUM tiles; memset
                    # once so the garbage lanes (16..31 of each quadrant)
                    # stay finite/zero forever.
                    xps = []
                    pss = []
                    for _ in range(2):
                        xp_t = xpp.tile([128, 512], f16)
                        nc.gpsimd.memset(xp_t[:], 0.0)
                        xps.append(xp_t)
                        ps = prp.tile([128, 512], f32, space="PSUM")
                        nc.vector.memset(ps[:], 0.0)
                        pss.append(ps)
                    for t in range(t_steps):
                        xp_t = xps[t % 2]
                        ps = pss[t % 2]
                        nc.gpsimd.dma_start(
                            xp_t[0:4 * BL, :], xp_src[t, :, :])
                        # round 0: inject xp into PSUM via identity-select
                        # matmuls (P_j[p,b] = [p == 16j+b]); rounds 1..16
                        # accumulate the recurrence on top. Col-tiled: group
                        # j streams W n-chunk j; the four 512-col streams
                        # run concurrently on the PE.
                        for j in range(NT):
                            nc.tensor.matmul(
                                ps[32 * j:32 * j + BL, :],
                                lhsT=ident[:, BL * j:BL * j + BL],
                                rhs=xp_t[:, :],
                                start=True, stop=False,
                                tile_position=(0, 32 * j),
                                skip_group_check=True,
                            )
                        for k in range(KT):
                            for j in range(NT):
                                nc.tensor.matmul(
                                    ps[32 * j:32 * j + BL, :],
                                    lhsT=hT_sb[:, k * 32:k * 32 + BL],
                                    rhs=W_sb[:, (k * NT + j) * 512:(k * NT + j + 1) * 512],
                                    start=False, stop=(k == KT - 1),
                                    tile_position=(0, 32 * j),
                                    skip_group_check=True,
                                )
                        hT_next = stp.tile([128, KT * 32], f16, tag="hT")
                        # tanh + single DVE 32x32 block-transpose (each block
                        # transposes in place within its own partition
                        # quadrant; W rows+cols are host-permuted so this
                        # produces the k-slot layout directly), split in
                        # column halves: the first half unlocks next step's
                        # k-rounds 0..7 early.
                        for hh in range(2):
                            cs = slice(hh * 256, hh * 256 + 256)
                            nc.scalar.activation(h_sb[:, cs], ps[:, cs], Tanh)
                            nc.vector.transpose(hT_next[:, cs], h_sb[:, cs])
                        if layer == 0:
                            nc.gpsimd.dma_start(
                                y0T_d[t, :, :],
                                hT_next[:, :]
                                .rearrange("p (k b2) -> p k b2", k=KT)
                                [:, :, 0:BL])
                        else:
                            nc.gpsimd.dma_start(
                                h1_d[t * 128:(t + 1) * 128, :], h_sb[:])
                        hT_sb = hT_next
                        if interleave is not None:
                            interleave(t)

            # -- Phases B+C+D: recurrences with xp1 GEMM spread over both --
            load_w(W_sb, w_hh0, KT)
            W2_sb = wp.tile([128, KT * H], f16)
            load_w(W2_sb, w_ih1, KT)
            bias2_sb = st.tile([128, H], f32)
            load_bias(bias2_sb, b1)
            with (
                tc.tile_pool(name="lh", bufs=2) as lhp,
                tc.tile_pool(name="pc", bufs=2, space="PSUM") as pcp,
                tc.tile_pool(name="oc", bufs=4) as ocp,
            ):
                lh_tiles = {}

                def load_chunk(c):
                    # single contiguous [p, t, 256] DMA (512B runs) into a
                    # t-major staging tile, then one DVE re-tile copy into
                    # the k-major layout the stationary matmul operand needs.
                    lh_raw = lhp.tile([128, H], f16, tag="lhr")
                    nc.gpsimd.dma_start(
                        lh_raw[:, :],
                        y0T_d[c * CH:(c + 1) * CH, :, :]
                        .rearrange("t p c2 -> p t c2"),
                    )
                    lh = lhp.tile([128, H], f16, tag="lh")
                    lh_tiles[c] = lh
                    nc.vector.tensor_copy(
                        lh[:, :].rearrange("p (kk t b2) -> p kk t b2",
                                           kk=KT, t=CH),
                        lh_raw[:, :].rearrange("p (t kk b2) -> p kk t b2",
                                               t=CH, kk=KT),
                    )

                def emit_part(c, n):
                    # one n-chunk of xp1 chunk c: 16 full-width M=128
                    # matmuls filling the recurrence's per-step PE gaps.
                    lh = lh_tiles[c]
                    ps = pcp.tile([128, 512], f32, space="PSUM")
                    for k in range(KT):
                        nc.tensor.matmul(
                            ps[:],
                            lhsT=lh[:, k * 128:(k + 1) * 128],
                            rhs=W2_sb[:, (k * NT + n) * 512:(k * NT + n + 1) * 512],
                            start=(k == 0), stop=(k == KT - 1),
                        )
                    oc = ocp.tile([128, 512], f16)
                    nc.vector.tensor_add(
                        oc[:], ps[:], bias2_sb[:, n * 512:(n + 1) * 512])
                    nc.gpsimd.dma_start(
                        xp1_d[c * CH:(c + 1) * CH, BL * n:BL * n + BL, :],
                        oc[:])
                    if n == NT - 1:
                        del lh_tiles[c]

                # schedules: part g = 4c+n fires after a given step; loads
                # prefetch a few steps ahead. rec0 carries parts while y0T
                # becomes available (t >= 8c+15); rec1 carries the rest at a
                # cadence that stays ahead of its own xp1 consumption.
                nparts = nchunk * NT
                acts0 = {}
                acts1 = {}
                g0 = 0
                for g in range(nparts):
                    c, n = divmod(g, NT)
                    t_fire = max(2 * CH - 1 + 8 * c, 15 + 4 * g) \
                        if t_steps >= 256 else (2 * CH - 1 + 8 * c + n)
                    if t_fire < t_steps:
                        acts0.setdefault(t_fire, []).append(("part", c, n))
                        g0 = g + 1
                    else:
                        break
                pre_parts = []
                for g in range(g0, nparts):
                    c, n = divmod(g, NT)
                    t_fire = 2 + 3 * (g - g0)
                    if t_steps >= 256 and t_fire <= 8 * c - 1:
                        acts1.setdefault(t_fire, []).append(("part", c, n))
                    else:
                        pre_parts.append((c, n))
                # chunk loads: 4 steps before the chunk's first part
                firstpart = {}
                for acts in (acts0, acts1):
                    for t_fire, lst in acts.items():
                        for (_, c, n) in lst:
                            if n == 0:
                                firstpart[c] = (acts, t_fire)
                for c, (acts, t_fire) in firstpart.items():
                    tl = t_fire - 4
                    if acts is acts1 and c * CH + CH - 1 >= t_steps:
                        # produced only at the very end of rec0: load in rec1
                        tl = max(0, tl)
                    if tl < 0:
                        tl = 0
                    acts.setdefault(tl, [])
                    acts[tl].insert(0, ("load", c, 0))

                def mk_interleave(acts):
                    def interleave(t):
                        for act in acts.get(t, []):
                            if act[0] == "load":
                                load_chunk(act[1])
                            else:
                                emit_part(act[1], act[2])
                    return interleave

                recurrence(0, xp0_d, interleave=mk_interleave(acts0))
                load_w(W_sb, w_hh1, KT)
                with nc.named_scope("phaseC"):
                    for c, n in pre_parts:
                        if n == 0 and c not in lh_tiles:
                            load_chunk(c)
                        emit_part(c, n)
                recurrence(1, xp1_d, interleave=mk_interleave(acts1))

            # final capture: out row 16j+b = h1[b, chunk j perm cols] at t=len_b-1
            with tc.tile_pool(name="cap", bufs=1) as cp:
                ci = cp.tile([4 * BL, 1], i32)
                nc.gpsimd.dma_start(ci[:], cap_idx[:, :])
                og = cp.tile([4 * BL, 512], f16)
                nc.gpsimd.indirect_dma_start(
                    out=og[:], out_offset=None,
                    in_=h1_d[:],
                    in_offset=bass.IndirectOffsetOnAxis(ap=ci[:, :1], axis=0),
                )
                nc.gpsimd.dma_start(out_h[:, :], og[:])

    nc.finalize()
    return nc


def _install_ntff_hook():
    """The trimmed agent image lacks antenv.axon_hooks — provide the tiny
    get/set registry and install the ctypes NTFF hook so trace=True works."""
    import types

    if "antenv.axon_hooks" in sys.modules:
        return
    m = types.ModuleType("antenv.axon_hooks")
    _hook = [None]
    m.set_axon_ntff_profile_hook = lambda h: _hook.__setitem__(0, h)
    m.get_axon_ntff_profile_hook = lambda: _hook[0]
    sys.modules["antenv.axon_hooks"] = m
    import antenv
    antenv.axon_hooks = m
    try:
        from trn_agent_boot.trn_boot import _ntff_profile_via_ctypes
        hook = _ntff_profile_via_ctypes("/opt/axon/libaxon_pjrt.so")
        if hook is not None:
            m.set_axon_ntff_profile_hook(hook)
        import concourse.bass_utils as bu
        bu.upload_artifacts = lambda d: str(d)
    except Exception:
        pass


def _permute_cols(w):
    """Swap the (mm, q) 32-col block fields within each 512-col chunk of the
    last dim: position 128q+32mm+v of a chunk holds natural col 128mm+32q+v.
    Involution. Makes each DVE transpose source a contiguous [32,128] slice."""
    shp = w.shape
    wr = w.reshape(-1, H // 512, 4, 4, 32)
    return np.ascontiguousarray(
        wr.transpose(0, 1, 3, 2, 4).reshape(shp))


_ROW_PERM = None


def _row_perm():
    """Contraction-row order matching the hT slot layout produced by the
    in-quadrant DVE block transpose: W_sb slot ss, partition 32j+v holds
    natural h-dim 512j + 128*(ss%4) + 32*(ss//4) + v."""
    global _ROW_PERM
    if _ROW_PERM is None:
        idx = np.empty(H, np.int64)
        for ss in range(KT):
            for j in range(4):
                v = np.arange(32)
                idx[ss * 128 + 32 * j + v] = (
                    512 * j + 128 * (ss % 4) + 32 * (ss // 4) + v)
        _ROW_PERM = idx
    return _ROW_PERM


def _make_in_maps(tokens, lengths, emb, W_ih0, W_hh0, b0, W_ih1, W_hh1, b1, ts):
    rp = _row_perm()
    W_ih0 = _permute_cols(W_ih0).astype(np.float16)
    W_hh0 = _permute_cols(W_hh0[rp]).astype(np.float16)
    W_ih1 = _permute_cols(W_ih1[rp]).astype(np.float16)
    W_hh1 = _permute_cols(W_hh1[rp]).astype(np.float16)
    b0 = _permute_cols(b0)
    b1 = _permute_cols(b1)
    emb16 = np.ascontiguousarray(emb.astype(np.float16))
    in_maps = []
    for c in range(NC):
        tok_c = tokens[c * BL:(c + 1) * BL, :ts]          # [16, ts]
        flat = tok_c.T.reshape(-1)                        # t-major rows
        tokT = np.ascontiguousarray(flat.reshape(-1, 128).T)  # [128, mt]
        len_c = np.minimum(lengths[c * BL:(c + 1) * BL].astype(np.int64), ts)
        r = np.arange(4 * BL)
        cap = ((len_c[r % BL] - 1) * 128 + 32 * (r // BL)
               + (r % BL)).astype(np.int32)[:, None]
        in_maps.append({
            "tokT": tokT,
            "cap_idx": np.ascontiguousarray(cap),
            "emb": emb16,
            "w_ih0": W_ih0, "w_hh0": W_hh0, "b0": b0,
            "w_ih1": W_ih1, "w_hh1": W_hh1, "b1": b1,
        })
    return in_maps


def _assemble(out_h_core):
    # out row 16j+b, col 128q+32mm+v -> h[b, 512j+128mm+32q+v]
    return (np.asarray(out_h_core).astype(np.float32)
            .reshape(4, BL, 4, 4, 32)
            .transpose(1, 0, 3, 2, 4).reshape(BL, H))


def kernel(tokens, lengths, emb, W_ih0, W_hh0, b0, W_ih1, W_hh1, b1,
           _t_steps=T, _trace=False):
    from concourse.bass_utils import run_bass_kernel_spmd

    if _trace:
        _install_ntff_hook()

    tokens = np.asarray(tokens).astype(np.int32)
    lengths = np.asarray(lengths).astype(np.int32)
    emb = np.ascontiguousarray(np.asarray(emb, dtype=np.float32))
    W_ih0 = np.ascontiguousarray(np.asarray(W_ih0, dtype=np.float32))
    W_hh0 = np.ascontiguousarray(np.asarray(W_hh0, dtype=np.float32))
    W_ih1 = np.ascontiguousarray(np.asarray(W_ih1, dtype=np.float32))
    W_hh1 = np.ascontiguousarray(np.asarray(W_hh1, dtype=np.float32))
    b0 = np.ascontiguousarray(np.asarray(b0, dtype=np.float32).reshape(1, H))
    b1 = np.ascontiguousarray(np.asarray(b1, dtype=np.float32).reshape(1, H))

    ts = _t_steps
    if ts not in _CACHE:
        _CACHE[ts] = _build(ts)
    nc = _CACHE[ts]

    in_maps = _make_in_maps(tokens, lengths, emb, W_ih0, W_hh0, b0,
                            W_ih1, W_hh1, b1, ts)

    res = run_bass_kernel_spmd(nc, in_maps, list(range(NC)), trace=_trace)
    STATS["exec_time_ns"] = res.exec_time_ns
    STATS["mean_exec_time_ns"] = res.mean_exec_time_ns
    STATS["scope_times"] = res.per_core_scope_times
    out = np.concatenate(
        [_assemble(res.results[c]["out_h"]) for c in range(NC)], axis=0)
    return out.astype(np.float32)


# revision 8
# speedup vs baseline: 1.2510x; 1.0779x over previous
"""Trainium2 Bass kernel for a 2-layer masked (ragged) Elman RNN — v2.

Problem: tokens [128,512] -> emb lookup [B,T,1024] -> RNN(1024->2048) ->
RNN(2048->2048) -> final hidden of layer 1, with per-sequence lengths
freezing the hidden state at t >= len (packed-sequence semantics).

Strategy (8 NeuronCores, data-parallel over batch, 16 seqs/core):
  Phase A: embedding gather + bulk input projection xp0 = X@W_ih0 + b0
  Phase B: layer-0 recurrence (512 serial steps)
  Phase C: bulk xp1 = y0 @ W_ih1 + b1 in chunks of 8 timesteps
  Phase D: layer-1 recurrence, storing h1 per step to DRAM; final
           per-sequence capture via indirect gather at t = len-1.

v2 changes vs baseline:
- The recurrence matmuls have M=16 (batch per core), using only 16 of the
  PE's 128 columns. They are now issued to the four 32-column PE tile
  groups (tile_position=(0,32j)): group j computes output n-chunk j with
  the full k accumulation, so four 512-col W streams run concurrently.
  Col-tiling requires 16-bit operands (fp32r matmuls fail the ISA check),
  so the whole matmul datapath is fp16 (same 10-bit mantissa as TF32;
  PSUM accumulation stays fp32).
- Pre-activations land in one PSUM bank in a "grouped" layout: partition
  32j+b holds pre[b, perm(512j+n')]. W columns are host-permuted (swap of
  the two 32-col block fields within each 512 chunk) so the per-step
  hidden-state transposition becomes 16 contiguous [32,128] DVE 32x32
  stream-transposes — entirely off the PE.
- xp tensors are stored in DRAM pre-swizzled to the grouped layout.

The recurrences run unmasked: for t < len the unmasked state equals the
reference's frozen-state values, and the capture row only reads t=len-1.
"""

import sys

sys.path.insert(0, "/opt/trn_rl_repo")

import numpy as np

B, T, V, D, H = 128, 512, 32000, 1024, 2048
NC = 8
BL = B // NC          # 16 sequences per core
KT = H // 128         # 16 k-tiles of the hidden dim
NT = H // 512         # 4 n-tiles (PSUM bank width)
DKT = D // 128        # 8 k-tiles of the embedding dim
CH = 128 // BL        # 8 timesteps per phase-A/C GEMM tile

STATS = {}
_CACHE = {}


def _build(t_steps):
    import concourse.bass as bass
    import concourse.mybir as mybir
    import concourse.tile as tile
    from concourse import bacc
    from concourse.masks import make_identity

    f32 = mybir.dt.float32
    f16 = mybir.dt.float16
    i32 = mybir.dt.int32
    Tanh = mybir.ActivationFunctionType.Tanh

    mt = (t_steps * BL) // 128   # phase-A tile count
    nchunk = t_steps // CH       # phase-C chunk count

    nc = bacc.Bacc("TRN2", target_bir_lowering=False, debug=False, num_devices=NC)

    tokT = nc.dram_tensor("tokT", [128, mt], i32, kind="ExternalInput")
    cap_idx = nc.dram_tensor("cap_idx", [4 * BL, 1], i32, kind="ExternalInput")
    emb = nc.dram_tensor("emb", [V, D], f16, kind="ExternalInput")
    w_ih0 = nc.dram_tensor("w_ih0", [D, H], f16, kind="ExternalInput")
    w_hh0 = nc.dram_tensor("w_hh0", [H, H], f16, kind="ExternalInput")
    b0 = nc.dram_tensor("b0", [1, H], f32, kind="ExternalInput")
    w_ih1 = nc.dram_tensor("w_ih1", [H, H], f16, kind="ExternalInput")
    w_hh1 = nc.dram_tensor("w_hh1", [H, H], f16, kind="ExternalInput")
    b1 = nc.dram_tensor("b1", [1, H], f32, kind="ExternalInput")
    out_h = nc.dram_tensor("out_h", [4 * BL, 512], f16, kind="ExternalOutput")

    # compact grouped layout: [t, 16*j + b, n'] = value[b, perm(512*j + n')]
    xp0_d = nc.dram_tensor("xp0_d", [t_steps, 4 * BL, 512], f16)
    xp1_d = nc.dram_tensor("xp1_d", [t_steps, 4 * BL, 512], f16)
    # per-step transposed state, compacted slots: [t, p, 16*k + b]
    y0T_d = nc.dram_tensor("y0T_d", [t_steps, 128, 256], f16)
    # layer-1 h per step, grouped rows (32*j + b valid for b < 16)
    h1_d = nc.dram_tensor("h1_d", [t_steps * 128, 512], f16)

    def load_w(W_sb, wsrc, ktiles):
        # W_sb col block (k*NT+n)*512 holds wsrc[k*128:(k+1)*128, n*512:(n+1)*512]
        for k in range(ktiles):
            nc.gpsimd.dma_start(
                W_sb[:, k * H:(k + 1) * H],
                wsrc[k * 128:(k + 1) * 128, :],
            )

    def load_bias(bias_sb, bsrc):
        nc.gpsimd.dma_start(bias_sb[0:1, :], bsrc[0:1, :])
        nc.gpsimd.partition_broadcast(bias_sb[:], bias_sb[0:1, :])

    with tile.TileContext(nc) as tc:
        with (
            tc.tile_pool(name="wpool", bufs=1) as wp,
            tc.tile_pool(name="state", bufs=1) as st,
        ):
            W_sb = wp.tile([128, KT * H], f16)       # 64KB/partition
            ident = st.tile([128, 128], f16)
            make_identity(nc, ident[:])
            bias_sb = st.tile([128, H], f32)
            tokens_sb = st.tile([128, mt], i32)
            nc.gpsimd.dma_start(tokens_sb[:], tokT[:, :])

            # ---------------- Phase A: embed + xp0 ----------------
            load_w(W_sb, w_ih0, DKT)
            load_bias(bias_sb, b0)
            with (
                nc.named_scope("phaseA"),
                tc.tile_pool(name="ga", bufs=3) as gp,
                tc.tile_pool(name="xt", bufs=2) as xtp,
                tc.tile_pool(name="pa", bufs=2, space="PSUM") as pap,
                tc.tile_pool(name="pn", bufs=4, space="PSUM") as pnp,
                tc.tile_pool(name="ot", bufs=4) as otp,
            ):
                for jj in range(mt):
                    xg = gp.tile([128, D], f16)
                    nc.gpsimd.indirect_dma_start(
                        out=xg[:], out_offset=None,
                        in_=emb[:],
                        in_offset=bass.IndirectOffsetOnAxis(
                            ap=tokens_sb[:, jj:jj + 1], axis=0),
                    )
                    xt_ps = pap.tile([128, D], f16, space="PSUM")
                    for k in range(DKT):
                        nc.tensor.transpose(
                            xt_ps[:, k * 128:(k + 1) * 128],
                            xg[:, k * 128:(k + 1) * 128],
                            ident[:],
                        )
                    xt = xtp.tile([128, D], f16)
                    nc.vector.tensor_copy(xt[:], xt_ps[:])
                    for n in range(NT):
                        ps = pnp.tile([128, 512], f32, space="PSUM")
                        for k in range(DKT):
                            nc.tensor.matmul(
                                ps[:],
                                lhsT=xt[:, k * 128:(k + 1) * 128],
                                rhs=W_sb[:, (k * NT + n) * 512:(k * NT + n + 1) * 512],
                                start=(k == 0), stop=(k == DKT - 1),
                            )
                        ot = otp.tile([128, 512], f16)
                        nc.vector.tensor_add(
                            ot[:], ps[:], bias_sb[:, n * 512:(n + 1) * 512])
                        nc.gpsimd.dma_start(
                            xp0_d[jj * CH:(jj + 1) * CH,
                                  BL * n:BL * n + BL, :],
                            ot[:])

            # ---------------- recurrence phase builder ----------------
            def recurrence(layer, xp_src, interleave=None):
                with (
                    nc.named_scope(f"rec{layer}"),
                    tc.tile_pool(name=f"st{layer}", bufs=2) as stp,
                    tc.tile_pool(name=f"xp{layer}", bufs=2) as xpp,
                    tc.tile_pool(name=f"hb{layer}", bufs=1) as hbp,
                    tc.tile_pool(name=f"pr{layer}", bufs=2, space="PSUM") as prp,
                ):
                    hT_sb = stp.tile([128, KT * 32], f16, tag="hT")
                    nc.gpsimd.memset(hT_sb[:], 0.0)
                    h_sb = hbp.tile([128, 512], f16)
                    # persistent double-buffered xp and PSUM tiles; memset
                    # once so the garbage lanes (16..31 of each quadrant)
                    # stay finite/zero forever.
                    xps = []
                    pss = []
                    for _ in range(2):
                        xp_t = xpp.tile([128, 512], f16)
                        nc.gpsimd.memset(xp_t[:], 0.0)
                        xps.append(xp_t)
                        ps = prp.tile([128, 512], f32, space="PSUM")
                        nc.vector.memset(ps[:], 0.0)
                        pss.append(ps)
                    for t in range(t_steps):
                        xp_t = xps[t % 2]
                        ps = pss[t % 2]
                        nc.gpsimd.dma_start(
                            xp_t[0:4 * BL, :], xp_src[t, :, :])
                        # round 0: inject xp into PSUM via identity-select
                        # matmuls (P_j[p,b] = [p == 16j+b]); rounds 1..16
                        # accumulate the recurrence on top. Col-tiled: group
                        # j streams W n-chunk j; the four 512-col streams
                        # run concurrently on the PE.
                        for j in range(NT):
                            nc.tensor.matmul(
                                ps[32 * j:32 * j + BL, :],
                                lhsT=ident[:, BL * j:BL * j + BL],
                                rhs=xp_t[:, :],
                                start=True, stop=False,
                                tile_position=(0, 32 * j),
                                skip_group_check=True,
                            )
                        for k in range(KT):
                            for j in range(NT):
                                nc.tensor.matmul(
                                    ps[32 * j:32 * j + BL, :],
                                    lhsT=hT_sb[:, k * 32:k * 32 + BL],
                                    rhs=W_sb[:, (k * NT + j) * 512:(k * NT + j + 1) * 512],
                                    start=False, stop=(k == KT - 1),
                                    tile_position=(0, 32 * j),
                                    skip_group_check=True,
                                )
                        hT_next = stp.tile([128, KT * 32], f16, tag="hT")
                        # tanh + single DVE 32x32 block-transpose (each block
                        # transposes in place within its own partition
                        # quadrant; W rows+cols are host-permuted so this
                        # produces the k-slot layout directly), split in
                        # column halves: the first half unlocks next step's
                        # k-rounds 0..7 early.
                        for hh in range(2):
                            cs = slice(hh * 256, hh * 256 + 256)
                            nc.scalar.activation(h_sb[:, cs], ps[:, cs], Tanh)
                            nc.vector.transpose(hT_next[:, cs], h_sb[:, cs])
                        if layer == 0:
                            nc.gpsimd.dma_start(
                                y0T_d[t, :, :],
                                hT_next[:, :]
                                .rearrange("p (k b2) -> p k b2", k=KT)
                                [:, :, 0:BL])
                        else:
                            nc.gpsimd.dma_start(
                                h1_d[t * 128:(t + 1) * 128, :], h_sb[:])
                        hT_sb = hT_next
                        if interleave is not None:
                            interleave(t)

            # -- Phases B+C+D: recurrences with xp1 GEMM spread over both --
            load_w(W_sb, w_hh0, KT)
            W2_sb = wp.tile([128, KT * H], f16)
            load_w(W2_sb, w_ih1, KT)
            bias2_sb = st.tile([128, H], f32)
            load_bias(bias2_sb, b1)
            with (
                tc.tile_pool(name="lh", bufs=2) as lhp,
                tc.tile_pool(name="pc", bufs=2, space="PSUM") as pcp,
                tc.tile_pool(name="oc", bufs=4) as ocp,
            ):
                lh_tiles = {}

                def load_chunk(c):
                    # single contiguous [p, t, 256] DMA (512B runs) into a
                    # t-major staging tile, then one DVE re-tile copy into
                    # the k-major layout the stationary matmul operand needs.
                    lh_raw = lhp.tile([128, H], f16, tag="lhr")
                    nc.gpsimd.dma_start(
                        lh_raw[:, :],
                        y0T_d[c * CH:(c + 1) * CH, :, :]
                        .rearrange("t p c2 -> p t c2"),
                    )
                    lh = lhp.tile([128, H], f16, tag="lh")
                    lh_tiles[c] = lh
                    nc.vector.tensor_copy(
                        lh[:, :].rearrange("p (kk t b2) -> p kk t b2",
                                           kk=KT, t=CH),
                        lh_raw[:, :].rearrange("p (t kk b2) -> p kk t b2",
                                               t=CH, kk=KT),
                    )

                def emit_part(c, n):
                    # one n-chunk of xp1 chunk c: 16 full-width M=128
                    # matmuls filling the recurrence's per-step PE gaps.
                    lh = lh_tiles[c]
                    ps = pcp.tile([128, 512], f32, space="PSUM")
                    for k in range(KT):
                        nc.tensor.matmul(
                            ps[:],
                            lhsT=lh[:, k * 128:(k + 1) * 128],
                            rhs=W2_sb[:, (k * NT + n) * 512:(k * NT + n + 1) * 512],
                            start=(k == 0), stop=(k == KT - 1),
                        )
                    oc = ocp.tile([128, 512], f16)
                    nc.vector.tensor_add(
                        oc[:], ps[:], bias2_sb[:, n * 512:(n + 1) * 512])
                    nc.gpsimd.dma_start(
                        xp1_d[c * CH:(c + 1) * CH, BL * n:BL * n + BL, :],
                        oc[:])
                    if n == NT - 1:
                        del lh_tiles[c]

                # schedules: part g = 4c+n fires after a given step; loads
                # prefetch a few steps ahead. rec0 carries parts while y0T
                # becomes available (t >= 8c+15); rec1 carries the rest at a
                # cadence that stays ahead of its own xp1 consumption.
                nparts = nchunk * NT
                acts0 = {}
                acts1 = {}
                g0 = 0
                for g in range(nparts):
                    c, n = divmod(g, NT)
                    t_fire = max(8 * c + 8, 15 + 3 * g) \
                        if t_steps >= 256 else (2 * CH - 1 + 8 * c + n)
                    if t_fire < t_steps:
                        acts0.setdefault(t_fire, []).append(("part", c, n))
                        g0 = g + 1
                    else:
                        break
                pre_parts = []
                for g in range(g0, nparts):
                    c, n = divmod(g, NT)
                    t_fire = 2 + 5 * (g - g0)
                    if t_steps >= 256 and t_fire <= 8 * c - 1:
                        acts1.setdefault(t_fire, []).append(("part", c, n))
                    else:
                        pre_parts.append((c, n))
                # chunk loads: 4 steps before the chunk's first part
                firstpart = {}
                for acts in (acts0, acts1):
                    for t_fire, lst in acts.items():
                        for (_, c, n) in lst:
                            if n == 0:
                                firstpart[c] = (acts, t_fire)
                for c, (acts, t_fire) in firstpart.items():
                    tl = t_fire - 4
                    if acts is acts1 and c * CH + CH - 1 >= t_steps:
                        # produced only at the very end of rec0: load in rec1
                        tl = max(0, tl)
                    if tl < 0:
                        tl = 0
                    acts.setdefault(tl, [])
                    acts[tl].insert(0, ("load", c, 0))

                def mk_interleave(acts):
                    def interleave(t):
                        for act in acts.get(t, []):
                            if act[0] == "load":
                                load_chunk(act[1])
                            else:
                                emit_part(act[1], act[2])
                    return interleave

                recurrence(0, xp0_d, interleave=mk_interleave(acts0))
                load_w(W_sb, w_hh1, KT)
                with nc.named_scope("phaseC"):
                    for c, n in pre_parts:
                        if n == 0 and c not in lh_tiles:
                            load_chunk(c)
                        emit_part(c, n)
                recurrence(1, xp1_d, interleave=mk_interleave(acts1))

            # final capture: out row 16j+b = h1[b, chunk j perm cols] at t=len_b-1
            with tc.tile_pool(name="cap", bufs=1) as cp:
                ci = cp.tile([4 * BL, 1], i32)
                nc.gpsimd.dma_start(ci[:], cap_idx[:, :])
                og = cp.tile([4 * BL, 512], f16)
                nc.gpsimd.indirect_dma_start(
                    out=og[:], out_offset=None,
                    in_=h1_d[:],
                    in_offset=bass.IndirectOffsetOnAxis(ap=ci[:, :1], axis=0),
                )
                nc.gpsimd.dma_start(out_h[:, :], og[:])

    nc.finalize()
    return nc


def _install_ntff_hook():
    """The trimmed agent image lacks antenv.axon_hooks — provide the tiny
    get/set registry and install the ctypes NTFF hook so trace=True works."""
    import types

    if "antenv.axon_hooks" in sys.modules:
        return
    m = types.ModuleType("antenv.axon_hooks")
    _hook = [None]
    m.set_axon_ntff_profile_hook = lambda h: _hook.__setitem__(0, h)
    m.get_axon_ntff_profile_hook = lambda: _hook[0]
    sys.modules["antenv.axon_hooks"] = m
    import antenv
    antenv.axon_hooks = m
    try:
        from trn_agent_boot.trn_boot import _ntff_profile_via_ctypes
        hook = _ntff_profile_via_ctypes("/opt/axon/libaxon_pjrt.so")
        if hook is not None:
            m.set_axon_ntff_profile_hook(hook)
        import concourse.bass_utils as bu
        bu.upload_artifacts = lambda d: str(d)
    except Exception:
        pass


def _permute_cols(w):
    """Swap the (mm, q) 32-col block fields within each 512-col chunk of the
    last dim: position 128q+32mm+v of a chunk holds natural col 128mm+32q+v.
    Involution. Makes each DVE transpose source a contiguous [32,128] slice."""
    shp = w.shape
    wr = w.reshape(-1, H // 512, 4, 4, 32)
    return np.ascontiguousarray(
        wr.transpose(0, 1, 3, 2, 4).reshape(shp))


_ROW_PERM = None


def _row_perm():
    """Contraction-row order matching the hT slot layout produced by the
    in-quadrant DVE block transpose: W_sb slot ss, partition 32j+v holds
    natural h-dim 512j + 128*(ss%4) + 32*(ss//4) + v."""
    global _ROW_PERM
    if _ROW_PERM is None:
        idx = np.empty(H, np.int64)
        for ss in range(KT):
            for j in range(4):
                v = np.arange(32)
                idx[ss * 128 + 32 * j + v] = (
                    512 * j + 128 * (ss % 4) + 32 * (ss // 4) + v)
        _ROW_PERM = idx
    return _ROW_PERM


def _make_in_maps(tokens, lengths, emb, W_ih0, W_hh0, b0, W_ih1, W_hh1, b1, ts):
    rp = _row_perm()
    W_ih0 = _permute_cols(W_ih0).astype(np.float16)
    W_hh0 = _permute_cols(W_hh0[rp]).astype(np.float16)
    W_ih1 = _permute_cols(W_ih1[rp]).astype(np.float16)
    W_hh1 = _permute_cols(W_hh1[rp]).astype(np.float16)
    b0 = _permute_cols(b0)
    b1 = _permute_cols(b1)
    emb16 = np.ascontiguousarray(emb.astype(np.float16))
    in_maps = []
    for c in range(NC):
        tok_c = tokens[c * BL:(c + 1) * BL, :ts]          # [16, ts]
        flat = tok_c.T.reshape(-1)                        # t-major rows
        tokT = np.ascontiguousarray(flat.reshape(-1, 128).T)  # [128, mt]
        len_c = np.minimum(lengths[c * BL:(c + 1) * BL].astype(np.int64), ts)
        r = np.arange(4 * BL)
        cap = ((len_c[r % BL] - 1) * 128 + 32 * (r // BL)
               + (r % BL)).astype(np.int32)[:, None]
        in_maps.append({
            "tokT": tokT,
            "cap_idx": np.ascontiguousarray(cap),
            "emb": emb16,
            "w_ih0": W_ih0, "w_hh0": W_hh0, "b0": b0,
            "w_ih1": W_ih1, "w_hh1": W_hh1, "b1": b1,
        })
    return in_maps


def _assemble(out_h_core):
    # out row 16j+b, col 128q+32mm+v -> h[b, 512j+128mm+32q+v]
    return (np.asarray(out_h_core).astype(np.float32)
            .reshape(4, BL, 4, 4, 32)
            .transpose(1, 0, 3, 2, 4).reshape(BL, H))


def kernel(tokens, lengths, emb, W_ih0, W_hh0, b0, W_ih1, W_hh1, b1,
           _t_steps=T, _trace=False):
    from concourse.bass_utils import run_bass_kernel_spmd

    if _trace:
        _install_ntff_hook()

    tokens = np.asarray(tokens).astype(np.int32)
    lengths = np.asarray(lengths).astype(np.int32)
    emb = np.ascontiguousarray(np.asarray(emb, dtype=np.float32))
    W_ih0 = np.ascontiguousarray(np.asarray(W_ih0, dtype=np.float32))
    W_hh0 = np.ascontiguousarray(np.asarray(W_hh0, dtype=np.float32))
    W_ih1 = np.ascontiguousarray(np.asarray(W_ih1, dtype=np.float32))
    W_hh1 = np.ascontiguousarray(np.asarray(W_hh1, dtype=np.float32))
    b0 = np.ascontiguousarray(np.asarray(b0, dtype=np.float32).reshape(1, H))
    b1 = np.ascontiguousarray(np.asarray(b1, dtype=np.float32).reshape(1, H))

    ts = _t_steps
    if ts not in _CACHE:
        _CACHE[ts] = _build(ts)
    nc = _CACHE[ts]

    in_maps = _make_in_maps(tokens, lengths, emb, W_ih0, W_hh0, b0,
                            W_ih1, W_hh1, b1, ts)

    res = run_bass_kernel_spmd(nc, in_maps, list(range(NC)), trace=_trace)
    STATS["exec_time_ns"] = res.exec_time_ns
    STATS["mean_exec_time_ns"] = res.mean_exec_time_ns
    STATS["scope_times"] = res.per_core_scope_times
    out = np.concatenate(
        [_assemble(res.results[c]["out_h"]) for c in range(NC)], axis=0)
    return out.astype(np.float32)
